# revision 19
# baseline (speedup 1.0000x reference)
"""Causal self-attention (B=2, T=2048, C=1024, H=16) on 8 Trainium2 NeuronCores.

Sharding (Megatron-style, per hint): core c handles batch b = c//4 and head
group g = c%4 (4 heads each).  c_attn is column-parallel (each core gets the
3x256 q/k/v columns for its heads), c_proj is row-parallel (each core gets
the 256 rows for its heads); the 4 partial outputs per batch are summed on
the host (the row-parallel all-reduce), plus b_proj.  Partials ship as bf16.

Single fused software-pipelined stream per core (keeps the PE HAM clock-gate
warm by never letting the Tensor engine idle during the ScalarE-bound
softmax phase):

  1. qkv projection for token block t4 is emitted as *filler* inside the
     attention j-loop of block t4-1, and the output projection of row block
     ib-1 likewise fills the attention of ib.  x^T is DMA'd in (co, t4)
     chunks so the first matmul starts ~2MB into the transfer.
  2. ST pair tiles [j, 2hp x i] live in one 2-bank PSUM tile so ONE ScalarE
     ACTIVATE (exp, fused 1/sqrt(D) scale) covers both heads of a pair --
     halves the fixed 352-cycle ACTIVATE overhead.  Causal masking is a
     multiplicative bf16 0/1 triangle on the post-exp tile (cheap DVE op).
  3. The two ST matmuls of a pair use K=64 row-halves of the PE array
     (distinct tile_position row groups) so they stream concurrently.
  4. YT[e, i] += V_aug^T @ P accumulates both heads into one 2-bank PSUM
     tile; row 64 is the softmax denominator (ones column of V_aug).
  5. Normalization: DVE reciprocal of the two denominator rows -> one bf16
     row, broadcast across 64 partitions by a K=1 PE matmul (two concurrent
     column-tiles), then a fused PSUMxPSUM multiply writes normalized Y^T
     straight to SBUF.  No DRAM round-trip.
"""

import os
import sys
import types
from collections import deque
from contextlib import ExitStack

import ml_dtypes
import numpy as np

for _p in ("/opt/trn_rl_repo",):
    if os.path.isdir(_p) and _p not in sys.path:
        sys.path.append(_p)
os.environ.setdefault("JAX_PLATFORMS", "cpu")

import concourse.bass as bass
import concourse.tile as tile
from concourse import bacc, mybir
from concourse.bass_utils import run_bass_kernel_spmd

B, T, C, H = 2, 2048, 1024, 16
P = 128
CO = C // P          # 8 contraction blocks for the qkv projection
HL = H // 4          # 4 local heads per core
D = C // H           # 64
F32 = mybir.dt.float32
BF16 = mybir.dt.bfloat16
EXPF = mybir.ActivationFunctionType.Exp
ADD = mybir.AluOpType.add
MULT = mybir.AluOpType.mult

_CACHE = {}


def _install_ntff_hook():
    """Agent image's antenv lacks axon_hooks; recreate so trace=True works."""
    try:
        from antenv import axon_hooks  # noqa: F401
        return
    except ImportError:
        pass
    try:
        import antenv
        from trn_agent_boot.trn_boot import _ntff_profile_via_ctypes
    except ImportError:
        return
    mod = types.ModuleType("antenv.axon_hooks")
    _hook = [None]
    mod.set_axon_ntff_profile_hook = lambda h: _hook.__setitem__(0, h)
    mod.get_axon_ntff_profile_hook = lambda: _hook[0]
    sys.modules["antenv.axon_hooks"] = mod
    antenv.axon_hooks = mod
    so = "/opt/axon/libaxon_pjrt.so"
    if os.path.exists(so):
        mod.set_axon_ntff_profile_hook(_ntff_profile_via_ctypes(so))


def build_module():
    nc = bacc.Bacc("TRN2", target_bir_lowering=False, debug=False, num_devices=8)

    # pre-swizzled on host so each DMA is one instruction with contiguous
    # per-partition lines: xt [p, t4, co, 512], w* [p, co, d], wp [p, ho, n]
    xt_d = nc.dram_tensor("xt", [P, T // 512, CO, 512], BF16,
                          kind="ExternalInput").ap()
    wq_d = nc.dram_tensor("wq", [P, CO, 256], BF16, kind="ExternalInput").ap()
    wk_d = nc.dram_tensor("wk", [P, CO, 256], BF16, kind="ExternalInput").ap()
    wv_d = nc.dram_tensor("wv", [P, CO, 256], BF16, kind="ExternalInput").ap()
    wp_d = nc.dram_tensor("wp", [P, 2, C], BF16, kind="ExternalInput").ap()
    bq_d = nc.dram_tensor("bq", [256], F32, kind="ExternalInput").ap()
    bk_d = nc.dram_tensor("bk", [256], F32, kind="ExternalInput").ap()
    bv_d = nc.dram_tensor("bv", [256], F32, kind="ExternalInput").ap()
    tri_d = nc.dram_tensor("tri", [P, 2 * P], BF16, kind="ExternalInput").ap()
    out_d = nc.dram_tensor("out", [T, C], BF16, kind="ExternalOutput").ap()

    NB = T // 512                       # 4 i-blocks of 512

    with tile.TileContext(nc) as tc, ExitStack() as ctx:
        const = ctx.enter_context(tc.tile_pool(name="const", bufs=1))
        s1w = ctx.enter_context(tc.tile_pool(name="s1w", bufs=1))
        # PSUM: 8 banks of [128, 512]f32.  psA(2) + psS(2x2) + psY(2) = 8.
        psA = ctx.enter_context(tc.tile_pool(name="psA", bufs=2, space="PSUM"))
        psS = ctx.enter_context(tc.tile_pool(name="psS", bufs=2, space="PSUM"))
        psY = ctx.enter_context(tc.tile_pool(name="psY", bufs=1, space="PSUM"))
        ppool = ctx.enter_context(tc.tile_pool(name="ppool", bufs=3))
        drp = ctx.enter_context(tc.tile_pool(name="drp", bufs=2))
        opool = ctx.enter_context(tc.tile_pool(name="opool", bufs=3))

        # ---- persistent SBUF tensors -------------------------------------
        qt = const.tile([P, 2, T], BF16, tag="qt")     # [d, ho, t]; head pair per ho
        kt = const.tile([P, 2, T], BF16, tag="kt")
        vsb = const.tile([P, T // P, HL, 66], BF16, tag="vsb")  # [tp, to, l, 1|V|1]
        yt2 = const.tile([P, 2, T], BF16, tag="yt2")   # normalized Y^T
        wp_sb = const.tile([P, 2, C], BF16, tag="wp")
        tri_sb = const.tile([P, 2 * P], BF16, tag="tri")   # 0/1 causal pair mask
        ones64 = const.tile([1, D], BF16, tag="ones64")
        bq_sb = const.tile([P, 2], F32, tag="bq")
        bk_sb = const.tile([P, 2], F32, tag="bk")
        bv_sb = const.tile([P, 256], F32, tag="bv")

        xt_sb = s1w.tile([P, CO, T], BF16, tag="xt")
        wq_sb = s1w.tile([P, CO, 256], BF16, tag="wq")
        wk_sb = s1w.tile([P, CO, 256], BF16, tag="wk")
        wv_sb = s1w.tile([P, CO, 256], BF16, tag="wv")

        # ---- input DMA, ordered by first use -----------------------------
        nc.sync.dma_start(wq_sb[:], wq_d)
        nc.sync.dma_start(xt_sb[:, 0:4, 0:512], xt_d[:, 0, 0:4])
        nc.sync.dma_start(xt_sb[:, 4:8, 0:512], xt_d[:, 0, 4:8])
        nc.sync.dma_start(wk_sb[:], wk_d)
        nc.sync.dma_start(bq_sb[:], bq_d.rearrange("(do p) -> p do", p=P))
        nc.sync.dma_start(bk_sb[:], bk_d.rearrange("(do p) -> p do", p=P))
        nc.sync.dma_start(wv_sb[:], wv_d)
        nc.sync.dma_start(
            bv_sb[:],
            bass.AP(tensor=bv_d.tensor, offset=bv_d.offset,
                    ap=[[0, P]] + list(bv_d.ap)),
        )
        nc.sync.dma_start(tri_sb[:], tri_d)
        for t4 in range(1, NB):
            nc.sync.dma_start(xt_sb[:, :, t4 * 512:(t4 + 1) * 512], xt_d[:, t4])
        nc.sync.dma_start(wp_sb[:], wp_d)
        nc.vector.memset(vsb[:, :, :, 65:66], 1.0)
        nc.vector.memset(ones64[:], 1.0)

        # ---- stage 1: qkv projection -------------------------------------
        def qk_group(w_sb, b_sb, dst, do, t4):
            # QT/KT d-major: psum[d, t] = W[:, dcols]^T @ x^T
            ps = psA.tile([P, 512], F32, tag="acc", name="qkps")
            for co in range(CO):
                nc.tensor.matmul(
                    ps[:],
                    lhsT=w_sb[:, co, do * P:(do + 1) * P],
                    rhs=xt_sb[:, co, t4 * 512:(t4 + 1) * 512],
                    start=(co == 0), stop=(co == CO - 1),
                )
            nc.vector.tensor_scalar_add(
                dst[:, do, t4 * 512:(t4 + 1) * 512], ps[:], b_sb[:, do:do + 1])

        def v_group(to):
            # V t-major: psum[t, d] = x^T-block^T @ Wv
            ps = psA.tile([P, 512], F32, tag="acc", name="vps")[:, 0:256]
            for co in range(CO):
                nc.tensor.matmul(
                    ps[:],
                    lhsT=xt_sb[:, co, to * P:(to + 1) * P],
                    rhs=wv_sb[:, co, :],
                    start=(co == 0), stop=(co == CO - 1),
                )
            nc.vector.tensor_tensor(
                vsb[:, to, :, 1:65],
                ps[:].rearrange("p (l e) -> p l e", l=HL),
                bv_sb[:].rearrange("p (l e) -> p l e", l=HL),
                op=ADD,
            )

        def qk_emitters(t4):
            ems = []
            for do in range(2):
                ems.append(lambda do=do, t4=t4: qk_group(wq_sb, bq_sb, qt, do, t4))
                ems.append(lambda do=do, t4=t4: qk_group(wk_sb, bk_sb, kt, do, t4))
            return ems

        def v_emitters(t4):
            return [lambda to=to: v_group(to) for to in range(4 * t4, 4 * t4 + 4)]

        # ---- stage 5: output projection (row-parallel partial) -----------
        otiles = {}

        def proj_group(i1, nh):
            isl = slice(i1 * P, (i1 + 1) * P)
            nsl = slice(nh * 512, (nh + 1) * 512)
            ps = psA.tile([P, 512], F32, tag="acc", name="prps")
            for ho in range(2):
                nc.tensor.matmul(
                    ps[:], lhsT=yt2[:, ho, isl], rhs=wp_sb[:, ho, nsl],
                    start=(ho == 0), stop=(ho == 1))
            if nh == 0:
                otiles[i1] = opool.tile([P, C], BF16, tag="ot", name="ot")
            ot = otiles[i1]
            nc.vector.tensor_copy(ot[:, nsl], ps[:])
            if nh == 1:
                nc.sync.dma_start(out_d[isl, :], otiles.pop(i1)[:])

        def proj_emitters(ib):
            return [lambda i1=i1, nh=nh: proj_group(i1, nh)
                    for i1 in range(4 * ib, 4 * ib + 4) for nh in range(2)]

        # ---- stages 2-4: attention for head pair ho, row block ib --------
        tri3 = tri_sb.rearrange("p (h n) -> p h n", h=2)

        def attention(ho, ib, sched, after_prologue=None):
            njb = 4 * ib + 4
            ytp = psY.tile([P, 1024], F32, tag="ytp")

            def win(jb):
                r = jb - 4 * ib
                i0 = jb * P if r >= 0 else ib * 512
                return r, i0, (ib + 1) * 512 - i0

            pts = {}

            def st_pair(jb):
                r, i0, N = win(jb)
                jsl = slice(jb * P, (jb + 1) * P)
                stp = psS.tile([P, 1024], F32, tag="stp")
                for hp in range(2):
                    pb = hp * 64
                    nc.tensor.matmul(
                        stp[:, hp * 512:hp * 512 + N],
                        lhsT=kt[pb:pb + 64, ho, jsl],
                        rhs=qt[pb:pb + 64, ho, i0:i0 + N],
                        start=True, stop=True)
                pt = ppool.tile([P, 2, 512], BF16, tag="pt")
                nc.scalar.activation(
                    pt[:, :, :N],
                    stp.rearrange("p (h n) -> p h n", h=2)[:, :, :N],
                    EXPF, scale=float(1.0 / np.sqrt(D)))
                if r >= 0:
                    nc.vector.tensor_tensor(
                        pt[:, :, 0:P], pt[:, :, 0:P], tri3[:], op=MULT)
                pts[jb] = pt

            def yt_pair(jb):
                _, i0, N = win(jb)
                f0 = i0 - ib * 512
                last = jb == njb - 1
                pt = pts.pop(jb)
                for hp in range(2):
                    nc.tensor.matmul(
                        ytp[0:65, hp * 512 + f0:hp * 512 + f0 + N],
                        lhsT=vsb[:, jb, 2 * ho + hp, 1:66],
                        rhs=pt[:, hp, :N], start=(jb == 0), stop=last)

            st_pair(0)
            if njb > 1:
                st_pair(1)
            if after_prologue is not None:
                # previous block's denominator row -> SBUF (DVE), one filler
                # so the PE isn't stalled on it, then broadcast + normalize.
                # In a ho=0 prologue the filler may not be a proj group: the
                # pending normalize writes the yt2 rows proj reads.
                pre, post = after_prologue
                pre()
                sched.cover(allow_proj=(ho == 1))
                post()
            for jb in range(njb):
                if jb + 2 < njb:
                    st_pair(jb + 2)
                sched.step()
                yt_pair(jb)

            # normalize: denom row -> SBUF, K=1 matmul broadcasts it across
            # 64 partitions per hp (concurrent col-tiles), then one WIDE
            # fast reciprocal over all 128 partitions (the narrow
            # nc.vector.reciprocal on [1, 1024] costs 6.5us and stalls the
            # in-order PE queue behind the broadcast).
            dsb = drp.tile([1, 1024], BF16, tag="dsb")

            def norm_pre():
                nc.vector.tensor_copy(dsb[:], ytp[64:65, :])

            def norm_post():
                rps = psA.tile([P, 512], F32, tag="acc", name="rps")
                for hp in range(2):
                    nc.tensor.matmul(
                        rps[hp * 64:hp * 64 + 64, :],
                        lhsT=ones64[0:1, :],
                        rhs=dsb[0:1, hp * 512:(hp + 1) * 512],
                        start=True, stop=True)
                rsb = drp.tile([P, 512], F32, tag="rsb")
                nc.vector.reciprocal_approx_fast(rsb[:], rps[:])
                iw = slice(ib * 512, (ib + 1) * 512)
                for hp in range(2):
                    nc.vector.tensor_tensor(
                        yt2[hp * 64:hp * 64 + 64, ho, iw],
                        ytp[0:64, hp * 512:(hp + 1) * 512],
                        rsb[hp * 64:hp * 64 + 64, :], op=MULT)

            return norm_pre, norm_post

        # ---- fused pipeline ----------------------------------------------
        class _Sched:
            """PE filler scheduler for one row block's two attention calls.

            V-projection groups pop eagerly (their consumers are this same
            block's late YTs); qk/proj groups are paced evenly across the
            block's jb slots so the late, ScalarE-bound blocks keep the PE
            busy enough that the HAM clock-gate never re-throttles."""

            def __init__(self, v_ems, qk_ems, pr_ems, slots):
                self.v = deque(v_ems)
                self.qk = deque(qk_ems)
                self.pr = deque(pr_ems)
                self.slots = max(slots, 1)
                self.total = len(self.qk) + len(self.pr)
                self.done = 0
                self.slot = 0

            def _pop(self, allow_proj=True):
                if self.v:
                    self.v.popleft()()
                    return True
                if self.qk:
                    self.qk.popleft()()
                    self.done += 1
                    return True
                if allow_proj and self.pr:
                    self.pr.popleft()()
                    self.done += 1
                    return True
                return False

            def cover(self, allow_proj):
                self._pop(allow_proj)

            def step(self):
                self.slot += 1
                if self.v:
                    self.v.popleft()()
                    return
                target = -(-self.total * self.slot // self.slots)
                while self.done < target and (self.qk or self.pr):
                    self._pop(True)

            def flush(self):
                while self._pop(True):
                    pass

        # minimal prologue (Q/K for head pair 0, V block 0) so the ScalarE
        # exp stream -- the pacing chain -- starts ~12us earlier; the rest
        # of t4=0's qkv becomes ib0 filler, consumed just-in-time.
        qk0 = qk_emitters(0)
        v0 = v_emitters(0)
        qk0[0]()
        qk0[1]()
        v0[0]()
        pending_norm = None
        for ib in range(NB):
            sched = _Sched(
                v_emitters(ib) if ib >= 1 else v0[1:],
                qk_emitters(ib + 1) if ib + 1 < NB else [],
                proj_emitters(ib - 1) if ib >= 1 else [],
                2 * (4 * ib + 4))
            if ib == 0:
                sched.qk.extendleft([qk0[3], qk0[2]])
                sched.total += 2
            for ho in range(2):
                pending_norm = attention(ho, ib, sched,
                                         after_prologue=pending_norm)
            sched.flush()
        pending_norm[0]()
        pending_norm[1]()
        for em in proj_emitters(NB - 1):
            em()

    nc.compile()
    return nc


def _get_module():
    if "nc" not in _CACHE:
        _CACHE["nc"] = build_module()
    return _CACHE["nc"]


def _make_in_maps(x, W_attn, b_attn, W_proj):
    tri1 = np.where(np.arange(P)[None, :] >= np.arange(P)[:, None],
                    np.float32(1.0), np.float32(0.0))
    bf = ml_dtypes.bfloat16
    tri = np.concatenate([tri1, tri1], axis=1).astype(bf)

    def swz_w(w):        # [C, d] -> [p, co, d]
        return np.ascontiguousarray(w.reshape(CO, P, -1).transpose(1, 0, 2))

    def swz_xt(xb):      # [T, C] -> x^T as [p, t4, co, 512]
        return np.ascontiguousarray(
            xb.T.reshape(CO, P, T // 512, 512).transpose(1, 2, 0, 3))

    in_maps = []
    for core in range(8):
        b, g = divmod(core, 4)
        cs = slice(g * 256, (g + 1) * 256)
        in_maps.append({
            "xt": swz_xt(np.asarray(x[b])).astype(bf),
            "wq": swz_w(W_attn[:, g * 256:(g + 1) * 256]).astype(bf),
            "wk": swz_w(W_attn[:, C + g * 256:C + (g + 1) * 256]).astype(bf),
            "wv": swz_w(W_attn[:, 2 * C + g * 256:2 * C + (g + 1) * 256]).astype(bf),
            "wp": np.ascontiguousarray(
                W_proj[cs, :].reshape(2, P, C).transpose(1, 0, 2)).astype(bf),
            "bq": np.ascontiguousarray(b_attn[cs]),
            "bk": np.ascontiguousarray(b_attn[C + g * 256:C + (g + 1) * 256]),
            "bv": np.ascontiguousarray(b_attn[2 * C + g * 256:2 * C + (g + 1) * 256]),
            "tri": tri,
        })
    return in_maps


def _gather(results, b_proj):
    y = np.empty((B, T, C), np.float32)
    for b in range(B):
        acc = results[4 * b]["out"].astype(np.float32)
        for g in range(1, 4):
            acc = acc + results[4 * b + g]["out"].astype(np.float32)
        y[b] = acc + b_proj[None, :].astype(np.float32)
    return y


def kernel(x, W_attn, b_attn, W_proj, b_proj, _trace=False):
    x = np.asarray(x, np.float32)
    W_attn = np.asarray(W_attn, np.float32)
    b_attn = np.asarray(b_attn, np.float32)
    W_proj = np.asarray(W_proj, np.float32)
    b_proj = np.asarray(b_proj, np.float32)

    nc = _get_module()
    in_maps = _make_in_maps(x, W_attn, b_attn, W_proj)
    kw = {}
    if _trace:
        _install_ntff_hook()
        kw = dict(trace=True)
    res = run_bass_kernel_spmd(nc, in_maps, core_ids=list(range(8)), **kw)
    out = _gather(res.results, b_proj)
    if _trace:
        return out, res
    return out
